# revision 14
# baseline (speedup 1.0000x reference)
"""Causal multi-head attention on 8 Trainium2 NeuronCores.

Sharding: core c -> batch (c // 4), head-group (c % 4) of 4 heads
(tensor-parallel over the 16 heads, data-parallel over batch=2).
Each core computes its 4 heads' contribution to the output projection;
the host sums the 4 per-head-group partials per batch (the "all-reduce")
and adds b_O.

v2 notes (vs v1 baseline):
  - x ships in *column-block-major* layout (blocks: seq cols 0:512,
    512:1536, 1536:2048; d-chunk-major inside a block) so each block is
    a fully-contiguous full-rate DMA, and the first attention tile's
    exp starts ~16us in instead of ~31us.
  - all input DMAs ride the single sync queue in strict FIFO priority
    order (weights/x needed first go first); output DMAs alternate
    gpsimd/sync queues and stay off the Scalar engine (saturated by exp).
  - W_O emission for query tile j follows attn(1, j) immediately.
  - every PSUM accumulation group owns a full 2KB bank (start=True
    zeroes the whole bank region, not just the written range).
  - h1's score matmul runs full-width on diagonal chunks so the exp
    never reads uninitialized PSUM.
"""

import os
import sys

for _p in ("/opt/trn_rl_repo", "/root/.axon_site/_ro/trn_rl_repo"):
    if os.path.isdir(_p) and _p not in sys.path:
        sys.path.append(_p)

import ml_dtypes
import numpy as np

import concourse.bacc as bacc
import concourse.mybir as mybir
import concourse.tile as tile
from concourse.bass_utils import run_bass_kernel_spmd

F32 = mybir.dt.float32
BF16 = mybir.dt.bfloat16

B = 2          # batch
S = 2048       # sequence length
DM = 1024      # d_model
DH = 64        # d_head
NHEAD = 16     # total heads
NH = 4         # heads per core
NPAIR = 2      # head pairs per core
DC = DM // 128   # d_model chunks of 128 -> 8
KC = S // 128    # key chunks of 128 -> 16
QT = S // 512    # query tiles of 512 -> 4

BLKS = (512, 1024, 512)          # seq column blocks (0:512, 512:1536, 1536:2048)
BLK0 = (0, 512, 1536)            # block start columns

SIM_SAFE = False   # True: initialize garbage stripes (needed for CoreSim runs)

# Set by test harness to capture HW profile; harmless defaults for grading.
TRACE = False
TRACE_DIR = None
LAST_EXEC_NS = None


def _blk(col):
    """Map a seq column to (block index, local column)."""
    for bi in (2, 1, 0):
        if col >= BLK0[bi]:
            return bi, col - BLK0[bi]
    raise ValueError(col)


def _build(with_bias: bool):
    nc = bacc.Bacc("TRN2", target_bir_lowering=False, debug=False)

    xbd = [
        nc.dram_tensor(f"xb{i}", [128, DC, BLKS[i]], BF16, kind="ExternalInput").ap()
        for i in range(3)
    ]
    wqd = nc.dram_tensor("wq", [128, NPAIR, DC, 128], BF16, kind="ExternalInput").ap()
    wkd = nc.dram_tensor("wk", [128, NPAIR, DC, 128], BF16, kind="ExternalInput").ap()
    wvd = nc.dram_tensor("wv", [128, NPAIR, DC, 128], BF16, kind="ExternalInput").ap()
    wod = nc.dram_tensor("wo", [128, NPAIR * DM], BF16, kind="ExternalInput").ap()
    maskd = nc.dram_tensor("mask", [128, 128], BF16, kind="ExternalInput").ap()
    if with_bias:
        bqd = nc.dram_tensor("bq", [1, NH * DH], BF16, kind="ExternalInput").ap()
        bkd = nc.dram_tensor("bk", [1, NH * DH], BF16, kind="ExternalInput").ap()
        bvd = nc.dram_tensor("bv", [1, NH * DH], BF16, kind="ExternalInput").ap()
    outT = nc.dram_tensor("outT", [DM, S], F32, kind="ExternalOutput").ap()

    with tile.TileContext(nc) as tc:
        with (
            tc.tile_pool(name="const", bufs=1) as cpool,
            tc.tile_pool(name="qk", bufs=1) as qkpool,
            tc.tile_pool(name="xt", bufs=1) as xtpool,
            tc.tile_pool(name="expS", bufs=3) as epool,
            tc.tile_pool(name="small", bufs=2) as spool,
            tc.tile_pool(name="zt", bufs=1) as ztpool,
            tc.tile_pool(name="out", bufs=4) as opool,
            tc.tile_pool(name="ps", bufs=1, space="PSUM") as psP,
        ):
            xt = [xtpool.tile([128, DC, BLKS[i]], BF16, name=f"xt{i}") for i in range(3)]
            wq_sb = cpool.tile([128, NPAIR, DC, 128], BF16, name="wq")
            wk_sb = cpool.tile([128, NPAIR, DC, 128], BF16, name="wk")
            wv_sb = cpool.tile([128, NPAIR, DC, 128], BF16, name="wv")
            wo_sb = cpool.tile([128, NPAIR, DM], BF16, name="wo")
            mask_sb = cpool.tile([128, 128], BF16, name="mask")
            ones_bf = cpool.tile([128, DH], BF16, name="ones_bf")
            nc.vector.memset(ones_bf[:, :], 1.0)
            if with_bias:
                ones512 = cpool.tile([128, 512], BF16, name="ones512")
                nc.vector.memset(ones512[:, :], 1.0)
                bq_sb = cpool.tile([128, NH * DH], BF16, name="bq")
                bk_sb = cpool.tile([128, NH * DH], BF16, name="bk")
                bv_sb = cpool.tile([128, NH * DH], BF16, name="bv")

            qt_sb = [qkpool.tile([128, S], BF16, name=f"qt{p}") for p in range(NPAIR)]
            kt_sb = [qkpool.tile([128, S], BF16, name=f"kt{p}") for p in range(NPAIR)]
            v_sb = qkpool.tile([128, KC, NH * DH], BF16, name="v")

            # ---- input DMAs: single sync queue = strict FIFO priority ----
            # full-tensor weight loads keep 4KB contiguous rows (full DMA rate)
            nc.sync.dma_start(wq_sb[:, :, :, :], wqd[:, :, :, :])
            nc.sync.dma_start(wk_sb[:, :, :, :], wkd[:, :, :, :])
            if with_bias:
                nc.sync.dma_start(bq_sb[0:1, :], bqd[:, :])
                nc.sync.dma_start(bk_sb[0:1, :], bkd[:, :])
                nc.sync.dma_start(bv_sb[0:1, :], bvd[:, :])
            nc.sync.dma_start(xt[0][:, 0:4, :], xbd[0][:, 0:4, :])
            nc.sync.dma_start(xt[0][:, 4:8, :], xbd[0][:, 4:8, :])
            nc.sync.dma_start(mask_sb[:, :], maskd[:, :])
            nc.sync.dma_start(wv_sb[:, :, :, :], wvd[:, :, :, :])
            nc.sync.dma_start(xt[1][:, :, 0:512], xbd[1][:, :, 0:512])
            nc.sync.dma_start(xt[1][:, :, 512:1024], xbd[1][:, :, 512:1024])
            nc.sync.dma_start(xt[2][:, :, :], xbd[2][:, :, :])
            nc.sync.dma_start(wo_sb[:, :, :], wod[:, :])

            def qk_proj(p, q):
                """bf16 projection of Q and K, pair p, query tile q."""
                bi, lo = _blk(q * 512)
                for pj in range(2):
                    ps = psP.tile([128, 512], F32, name="ps_acc", bufs=2)
                    w_sb = wq_sb if pj == 0 else wk_sb
                    for c in range(DC):
                        nc.tensor.matmul(
                            ps[:, :],
                            lhsT=w_sb[:, p, c, :],
                            rhs=xt[bi][:, c, lo:lo + 512],
                            start=(c == 0),
                            stop=(c == DC - 1 and not with_bias),
                        )
                    if with_bias:
                        bias_t = bq_sb if pj == 0 else bk_sb
                        nc.tensor.matmul(
                            ps[:, :],
                            lhsT=bias_t[0:1, p * 128:(p + 1) * 128],
                            rhs=ones512[0:1, :],
                            start=False,
                            stop=True,
                        )
                    dst = qt_sb[p] if pj == 0 else kt_sb[p]
                    nc.vector.tensor_copy(dst[:, q * 512:(q + 1) * 512], ps[:, :])

            def v_proj(ks, pairs):
                """bf16 V projection for k-chunks ks (<=2 per call; each k
                gets its own full PSUM bank).  pairs: (0,), (1,) or (0, 1)."""
                accs = [psP.tile([128, 512], F32, name="ps_acc", bufs=2) for _ in ks]
                n = 128 * len(pairs)
                for c in range(DC):
                    for i, k in enumerate(ks):
                        bi, lo = _blk(k * 128)
                        rhs = (wv_sb[:, pairs[0], c, :] if len(pairs) == 1
                               else wv_sb[:, :, c, :])
                        nc.tensor.matmul(
                            accs[i][:, 0:n],
                            lhsT=xt[bi][:, c, lo:lo + 128],
                            rhs=rhs,
                            start=(c == 0),
                            stop=(c == DC - 1 and not with_bias),
                            skip_group_check=True,
                        )
                if with_bias:
                    brhs = (bv_sb[0:1, pairs[0] * 128:(pairs[0] + 1) * 128]
                            if len(pairs) == 1 else bv_sb[0:1, :])
                    for i in range(len(ks)):
                        nc.tensor.matmul(
                            accs[i][:, 0:n],
                            lhsT=ones512[0:1, 0:128],
                            rhs=brhs,
                            start=False,
                            stop=True,
                            skip_group_check=True,
                        )
                for i, k in enumerate(ks):
                    dst = (v_sb[:, k, pairs[0] * 128:(pairs[0] + 1) * 128]
                           if len(pairs) == 1 else v_sb[:, k, :])
                    nc.vector.tensor_copy(dst, accs[i][:, 0:n])

            zts = {}  # (p, j) -> zt tile
            es_tiles = {}
            pv_ps = {}

            def attn_scores(p, j, r0, r1):
                if r0 == 0:
                    es_tiles[(p, j)] = epool.tile([128, KC * 2 * 512], BF16, name="es")
                es = es_tiles[(p, j)]
                for c in range(r0, r1):
                    tp = c - 4 * j
                    a = 128 * tp if tp >= 0 else 0
                    ps = psP.tile([128, 1024], F32, name="ps_sc", bufs=2)
                    for hi in range(2):
                        # in SIM_SAFE mode h1 runs full-width on diagonal
                        # chunks so the exp never reads uninitialized PSUM;
                        # on HW the stripe is dead garbage (never consumed)
                        ah = a if (hi == 0 or not SIM_SAFE) else 0
                        prow = slice(64 * hi, 64 * hi + 64)
                        nc.tensor.matmul(
                            ps[:, 512 * hi + ah:512 * (hi + 1)],
                            lhsT=kt_sb[p][prow, c * 128:(c + 1) * 128],
                            rhs=qt_sb[p][prow, j * 512 + ah:(j + 1) * 512],
                            start=True,
                            stop=True,
                        )
                    nc.scalar.activation(
                        es[:, c * 1024 + a:(c + 1) * 1024],
                        ps[:, a:],
                        mybir.ActivationFunctionType.Exp,
                    )
                    if tp >= 0:
                        for hi in range(2):
                            sl = slice((c * 2 + hi) * 512 + a,
                                       (c * 2 + hi) * 512 + a + 128)
                            nc.vector.tensor_mul(
                                out=es[:, sl], in0=es[:, sl], in1=mask_sb[:, :],
                            )

            def attn_pv(p, j, r0, r1):
                nck = 4 * (j + 1)
                es = es_tiles[(p, j)]
                # PV + column sums; h0/h1 col-packed, emitted adjacently so
                # they run concurrently in disjoint array column groups
                if r0 == 0:
                    pv_ps[(p, j)] = (
                        psP.tile([128, 512], F32, name="ps_zs", bufs=2),
                        psP.tile([128, 512], F32, name="ps_zs", bufs=2),
                    )
                ps_z2, ps_s2 = pv_ps[(p, j)]
                ps_z = [ps_z2, ps_z2]
                ps_s = [ps_s2, ps_s2]
                for c in range(r0, r1):
                    tp = c - 4 * j
                    a = 128 * tp if tp >= 0 else 0
                    for hi in range(2):
                        col = 64 * hi
                        hcore = 2 * p + hi
                        nc.tensor.matmul(
                            ps_z[hi][col:col + 64, a:512],
                            lhsT=v_sb[:, c, hcore * DH:(hcore + 1) * DH],
                            rhs=es[:, (c * 2 + hi) * 512 + a:(c * 2 + hi + 1) * 512],
                            start=(c == 0),
                            stop=(c == nck - 1),
                            tile_position=(0, col),
                            skip_group_check=True,
                        )
                    for hi in range(2):
                        col = 64 * hi
                        nc.tensor.matmul(
                            ps_s[hi][col:col + 64, a:512],
                            lhsT=ones_bf[:, :],
                            rhs=es[:, (c * 2 + hi) * 512 + a:(c * 2 + hi + 1) * 512],
                            start=(c == 0),
                            stop=(c == nck - 1),
                            tile_position=(0, col),
                            skip_group_check=True,
                        )
                if r1 == nck:
                    recip = spool.tile([128, 512], F32, name="recip")
                    nc.vector.reciprocal_approx_fast(recip[:, :], ps_s2[:, :])
                    zt = ztpool.tile([128, 512], BF16, name=f"zt{p}{j}")
                    nc.vector.tensor_mul(zt[:, :], ps_z2[:, :], recip[:, :])
                    zts[(p, j)] = zt

            def emit_wo(j, d0, d1):
                for d in range(d0, d1):
                    ps = psP.tile([128, 512], F32, name="ps_acc", bufs=2)
                    for p in range(NPAIR):
                        nc.tensor.matmul(
                            ps[:, :],
                            lhsT=wo_sb[:, p, d * 128:(d + 1) * 128],
                            rhs=zts[(p, j)][:, :],
                            start=(p == 0),
                            stop=(p == NPAIR - 1),
                        )
                    ot = opool.tile([128, 512], F32, name="ot")
                    if j == QT - 1 and d % 2 == 1:
                        nc.scalar.copy(ot[:, :], ps[:, :])
                    else:
                        nc.vector.tensor_copy(ot[:, :], ps[:, :])
                    eng = nc.gpsimd if d % 2 == 0 else nc.sync
                    eng.dma_start(
                        outT[d * 128:(d + 1) * 128, j * 512:(j + 1) * 512],
                        ot[:, :],
                    )

            # schedule: explicit interleave so the exp stream (Scalar) is
            # fed scores continuously while PV/projections/WO fill the PE,
            # ordered to chase the input DMA stream at the start.
            sc, pv = attn_scores, attn_pv
            qk_proj(0, 0)
            sc(0, 0, 0, 4)
            v_proj([0, 1], (0, 1))
            v_proj([2, 3], (0, 1))
            pv(0, 0, 0, 4)
            qk_proj(0, 1)
            sc(0, 1, 0, 2)
            sc(0, 1, 2, 4)
            v_proj([4, 5], (0, 1))
            sc(0, 1, 4, 6)
            v_proj([6, 7], (0, 1))
            sc(0, 1, 6, 8)
            pv(0, 1, 0, 4)
            qk_proj(0, 2)
            sc(0, 2, 0, 2)
            pv(0, 1, 4, 8)
            sc(0, 2, 2, 4)
            v_proj([8, 9], (0, 1))
            sc(0, 2, 4, 6)
            v_proj([10, 11], (0, 1))
            sc(0, 2, 6, 8)
            pv(0, 2, 0, 3)
            sc(0, 2, 8, 10)
            qk_proj(0, 3)
            sc(0, 2, 10, 12)
            pv(0, 2, 3, 6)
            sc(0, 3, 0, 2)
            pv(0, 2, 6, 9)
            sc(0, 3, 2, 4)
            pv(0, 2, 9, 12)
            sc(0, 3, 4, 6)
            v_proj([12, 13], (0, 1))
            sc(0, 3, 6, 8)
            v_proj([14, 15], (0, 1))
            sc(0, 3, 8, 10)
            qk_proj(1, 0)
            sc(0, 3, 10, 12)
            pv(0, 3, 0, 3)
            sc(0, 3, 12, 14)
            pv(0, 3, 3, 6)
            sc(0, 3, 14, 16)
            pv(0, 3, 6, 9)
            sc(1, 0, 0, 2)
            pv(0, 3, 9, 12)
            sc(1, 0, 2, 4)
            pv(0, 3, 12, 16)
            qk_proj(1, 1)
            sc(1, 1, 0, 2)
            pv(1, 0, 0, 2)
            sc(1, 1, 2, 4)
            pv(1, 0, 2, 4)
            sc(1, 1, 4, 6)
            qk_proj(1, 2)
            sc(1, 1, 6, 8)
            pv(1, 1, 0, 2)
            sc(1, 2, 0, 2)
            pv(1, 1, 2, 4)
            sc(1, 2, 2, 4)
            pv(1, 1, 4, 6)
            sc(1, 2, 4, 6)
            pv(1, 1, 6, 8)
            sc(1, 2, 6, 8)
            emit_wo(0, 0, 4)
            sc(1, 2, 8, 10)
            emit_wo(0, 4, 8)
            sc(1, 2, 10, 12)
            qk_proj(1, 3)
            sc(1, 3, 0, 2)
            pv(1, 2, 0, 2)
            sc(1, 3, 2, 4)
            pv(1, 2, 2, 4)
            sc(1, 3, 4, 6)
            pv(1, 2, 4, 6)
            sc(1, 3, 6, 8)
            pv(1, 2, 6, 8)
            sc(1, 3, 8, 10)
            pv(1, 2, 8, 10)
            sc(1, 3, 10, 12)
            pv(1, 2, 10, 12)
            sc(1, 3, 12, 14)
            emit_wo(1, 0, 4)
            sc(1, 3, 14, 16)
            emit_wo(1, 4, 8)
            pv(1, 3, 0, 4)
            emit_wo(2, 0, 4)
            pv(1, 3, 4, 8)
            emit_wo(2, 4, 8)
            pv(1, 3, 8, 12)
            pv(1, 3, 12, 16)
            emit_wo(3, 0, 8)

    nc.compile()
    return nc


_cache = {}


def _get(with_bias: bool):
    if with_bias not in _cache:
        _cache[with_bias] = _build(with_bias)
    return _cache[with_bias]


def kernel(x, W_Q, W_K, W_V, W_O, b_Q, b_K, b_V, b_O):
    global LAST_EXEC_NS
    x = np.asarray(x, dtype=np.float32)
    W_Q = np.asarray(W_Q, dtype=np.float32)
    W_K = np.asarray(W_K, dtype=np.float32)
    W_V = np.asarray(W_V, dtype=np.float32)
    W_O = np.asarray(W_O, dtype=np.float32)
    b_Q = np.asarray(b_Q, dtype=np.float32)
    b_K = np.asarray(b_K, dtype=np.float32)
    b_V = np.asarray(b_V, dtype=np.float32)
    b_O = np.asarray(b_O, dtype=np.float32)

    with_bias = bool(np.any(b_Q) or np.any(b_K) or np.any(b_V))
    nc = _get(with_bias)

    bf = ml_dtypes.bfloat16

    xT = np.ascontiguousarray(x.transpose(0, 2, 1))  # [B, DM, S]
    # chunk-major [B, 128, DC, S] bf16
    xc = np.ascontiguousarray(
        xT.reshape(B, DC, 128, S).transpose(0, 2, 1, 3)
    ).astype(bf)
    kp = np.arange(128)[:, None]
    qf = np.arange(128)[None, :]
    mask = np.where(qf >= kp, 1.0, 0.0).astype(bf)

    def pack_w(w):  # [DM, NH*DH] -> [128, NPAIR, DC*128] bf16, pair-major
        w = w.reshape(DC, 128, NPAIR, 128)           # [c, p, pair, m]
        w = w.transpose(1, 2, 0, 3)                  # [p, pair, c, m]
        return np.ascontiguousarray(w.reshape(128, NPAIR, DC, 128)).astype(bf)

    in_maps = []
    for core in range(8):
        b, g = divmod(core, 4)
        hs = slice(NH * g, NH * g + NH)

        m = {
            "wq": pack_w((W_Q[hs] * 0.125).transpose(1, 0, 2).reshape(DM, NH * DH)),
            "wk": pack_w(W_K[hs].transpose(1, 0, 2).reshape(DM, NH * DH)),
            "wv": pack_w(W_V[hs].transpose(1, 0, 2).reshape(DM, NH * DH)),
            "wo": np.ascontiguousarray(
                W_O[hs].reshape(NH * DH, DM).astype(bf)
                .reshape(NPAIR, 128, DM).transpose(1, 0, 2).reshape(128, NPAIR * DM)
            ),
            "mask": mask,
        }
        for i in range(3):
            m[f"xb{i}"] = np.ascontiguousarray(
                xc[b][:, :, BLK0[i]:BLK0[i] + BLKS[i]]
            )
        if with_bias:
            m["bq"] = (b_Q[hs] * 0.125).reshape(1, NH * DH).astype(bf)
            m["bk"] = b_K[hs].reshape(1, NH * DH).astype(bf)
            m["bv"] = b_V[hs].reshape(1, NH * DH).astype(bf)
        in_maps.append(m)

    kwargs = {}
    if TRACE:
        kwargs = {"trace": True}
        if TRACE_DIR:
            kwargs["tmpdir"] = TRACE_DIR
    res = run_bass_kernel_spmd(nc, in_maps, list(range(8)), **kwargs)
    LAST_EXEC_NS = res.exec_time_ns

    out = np.empty((B, S, DM), dtype=np.float32)
    for b in range(B):
        acc = res.results[4 * b]["outT"].astype(np.float32)
        for g in range(1, 4):
            acc = acc + res.results[4 * b + g]["outT"]
        out[b] = acc.T + b_O[None, :]
    return out


# revision 15
# speedup vs baseline: 1.0237x; 1.0237x over previous
"""Causal multi-head attention on 8 Trainium2 NeuronCores.

Sharding: core c -> batch (c // 4), head-group (c % 4) of 4 heads
(tensor-parallel over the 16 heads, data-parallel over batch=2).
Each core computes its 4 heads' contribution to the output projection;
the host sums the 4 per-head-group partials per batch (the "all-reduce")
and adds b_O.

v2 notes (vs v1 baseline):
  - x ships in *column-block-major* layout (blocks: seq cols 0:512,
    512:1536, 1536:2048; d-chunk-major inside a block) so each block is
    a fully-contiguous full-rate DMA, and the first attention tile's
    exp starts ~16us in instead of ~31us.
  - all input DMAs ride the single sync queue in strict FIFO priority
    order (weights/x needed first go first); output DMAs alternate
    gpsimd/sync queues and stay off the Scalar engine (saturated by exp).
  - W_O emission for query tile j follows attn(1, j) immediately.
  - every PSUM accumulation group owns a full 2KB bank (start=True
    zeroes the whole bank region, not just the written range).
  - h1's score matmul runs full-width on diagonal chunks so the exp
    never reads uninitialized PSUM.
"""

import os
import sys

for _p in ("/opt/trn_rl_repo", "/root/.axon_site/_ro/trn_rl_repo"):
    if os.path.isdir(_p) and _p not in sys.path:
        sys.path.append(_p)

import ml_dtypes
import numpy as np

import concourse.bacc as bacc
import concourse.mybir as mybir
import concourse.tile as tile
from concourse.bass_utils import run_bass_kernel_spmd

F32 = mybir.dt.float32
BF16 = mybir.dt.bfloat16

B = 2          # batch
S = 2048       # sequence length
DM = 1024      # d_model
DH = 64        # d_head
NHEAD = 16     # total heads
NH = 4         # heads per core
NPAIR = 2      # head pairs per core
DC = DM // 128   # d_model chunks of 128 -> 8
KC = S // 128    # key chunks of 128 -> 16
QT = S // 512    # query tiles of 512 -> 4

BLKS = (512, 1024, 512)          # seq column blocks (0:512, 512:1536, 1536:2048)
BLK0 = (0, 512, 1536)            # block start columns

SIM_SAFE = False   # True: initialize garbage stripes (needed for CoreSim runs)

# Set by test harness to capture HW profile; harmless defaults for grading.
TRACE = False
TRACE_DIR = None
LAST_EXEC_NS = None


def _blk(col):
    """Map a seq column to (block index, local column)."""
    for bi in (2, 1, 0):
        if col >= BLK0[bi]:
            return bi, col - BLK0[bi]
    raise ValueError(col)


def _build(with_bias: bool):
    nc = bacc.Bacc("TRN2", target_bir_lowering=False, debug=False)

    xbd = [
        nc.dram_tensor(f"xb{i}", [128, DC, BLKS[i]], BF16, kind="ExternalInput").ap()
        for i in range(3)
    ]
    wqd = nc.dram_tensor("wq", [128, NPAIR, DC, 128], BF16, kind="ExternalInput").ap()
    wkd = nc.dram_tensor("wk", [128, NPAIR, DC, 128], BF16, kind="ExternalInput").ap()
    wvd = nc.dram_tensor("wv", [128, NPAIR, DC, 128], BF16, kind="ExternalInput").ap()
    wod = nc.dram_tensor("wo", [128, NPAIR * DM], BF16, kind="ExternalInput").ap()
    maskd = nc.dram_tensor("mask", [128, 128], BF16, kind="ExternalInput").ap()
    if with_bias:
        bqd = nc.dram_tensor("bq", [1, NH * DH], BF16, kind="ExternalInput").ap()
        bkd = nc.dram_tensor("bk", [1, NH * DH], BF16, kind="ExternalInput").ap()
        bvd = nc.dram_tensor("bv", [1, NH * DH], BF16, kind="ExternalInput").ap()
    outT = nc.dram_tensor("outT", [DM, S], F32, kind="ExternalOutput").ap()

    with tile.TileContext(nc) as tc:
        with (
            tc.tile_pool(name="const", bufs=1) as cpool,
            tc.tile_pool(name="qk", bufs=1) as qkpool,
            tc.tile_pool(name="xt", bufs=1) as xtpool,
            tc.tile_pool(name="expS", bufs=3) as epool,
            tc.tile_pool(name="small", bufs=2) as spool,
            tc.tile_pool(name="zt", bufs=1) as ztpool,
            tc.tile_pool(name="out", bufs=4) as opool,
            tc.tile_pool(name="ps", bufs=1, space="PSUM") as psP,
        ):
            xt = [xtpool.tile([128, DC, BLKS[i]], BF16, name=f"xt{i}") for i in range(3)]
            wq_sb = cpool.tile([128, NPAIR, DC, 128], BF16, name="wq")
            wk_sb = cpool.tile([128, NPAIR, DC, 128], BF16, name="wk")
            wv_sb = cpool.tile([128, NPAIR, DC, 128], BF16, name="wv")
            wo_sb = cpool.tile([128, NPAIR, DM], BF16, name="wo")
            mask_sb = cpool.tile([128, 128], BF16, name="mask")
            ones_bf = cpool.tile([128, DH], BF16, name="ones_bf")
            nc.vector.memset(ones_bf[:, :], 1.0)
            if with_bias:
                ones512 = cpool.tile([128, 512], BF16, name="ones512")
                nc.vector.memset(ones512[:, :], 1.0)
                bq_sb = cpool.tile([128, NH * DH], BF16, name="bq")
                bk_sb = cpool.tile([128, NH * DH], BF16, name="bk")
                bv_sb = cpool.tile([128, NH * DH], BF16, name="bv")

            qt_sb = [qkpool.tile([128, S], BF16, name=f"qt{p}") for p in range(NPAIR)]
            kt_sb = [qkpool.tile([128, S], BF16, name=f"kt{p}") for p in range(NPAIR)]
            v_sb = qkpool.tile([128, KC, NH * DH], BF16, name="v")

            # ---- input DMAs: single sync queue = strict FIFO priority ----
            # full-tensor weight loads keep 4KB contiguous rows (full DMA rate)
            nc.sync.dma_start(xt[0][:, 0:4, :], xbd[0][:, 0:4, :])
            nc.sync.dma_start(wq_sb[:, :, :, :], wqd[:, :, :, :])
            nc.sync.dma_start(xt[0][:, 4:8, :], xbd[0][:, 4:8, :])
            nc.sync.dma_start(wk_sb[:, :, :, :], wkd[:, :, :, :])
            if with_bias:
                nc.sync.dma_start(bq_sb[0:1, :], bqd[:, :])
                nc.sync.dma_start(bk_sb[0:1, :], bkd[:, :])
                nc.sync.dma_start(bv_sb[0:1, :], bvd[:, :])
            nc.sync.dma_start(mask_sb[:, :], maskd[:, :])
            nc.sync.dma_start(wv_sb[:, :, :, :], wvd[:, :, :, :])
            nc.sync.dma_start(xt[1][:, 0:4, :], xbd[1][:, 0:4, :])
            nc.sync.dma_start(xt[1][:, 4:8, :], xbd[1][:, 4:8, :])
            nc.sync.dma_start(xt[2][:, :, :], xbd[2][:, :, :])
            nc.sync.dma_start(wo_sb[:, :, :], wod[:, :])

            def qk_proj(p, q):
                """bf16 projection of Q and K, pair p, query tile q."""
                bi, lo = _blk(q * 512)
                for pj in range(2):
                    ps = psP.tile([128, 512], F32, name="ps_acc", bufs=2)
                    w_sb = wq_sb if pj == 0 else wk_sb
                    for c in range(DC):
                        nc.tensor.matmul(
                            ps[:, :],
                            lhsT=w_sb[:, p, c, :],
                            rhs=xt[bi][:, c, lo:lo + 512],
                            start=(c == 0),
                            stop=(c == DC - 1 and not with_bias),
                        )
                    if with_bias:
                        bias_t = bq_sb if pj == 0 else bk_sb
                        nc.tensor.matmul(
                            ps[:, :],
                            lhsT=bias_t[0:1, p * 128:(p + 1) * 128],
                            rhs=ones512[0:1, :],
                            start=False,
                            stop=True,
                        )
                    dst = qt_sb[p] if pj == 0 else kt_sb[p]
                    nc.vector.tensor_copy(dst[:, q * 512:(q + 1) * 512], ps[:, :])

            def v_proj(ks, pairs):
                """bf16 V projection for k-chunks ks (<=2 per call; each k
                gets its own full PSUM bank).  pairs: (0,), (1,) or (0, 1)."""
                accs = [psP.tile([128, 512], F32, name="ps_acc", bufs=2) for _ in ks]
                n = 128 * len(pairs)
                for c in range(DC):
                    for i, k in enumerate(ks):
                        bi, lo = _blk(k * 128)
                        rhs = (wv_sb[:, pairs[0], c, :] if len(pairs) == 1
                               else wv_sb[:, :, c, :])
                        nc.tensor.matmul(
                            accs[i][:, 0:n],
                            lhsT=xt[bi][:, c, lo:lo + 128],
                            rhs=rhs,
                            start=(c == 0),
                            stop=(c == DC - 1 and not with_bias),
                            skip_group_check=True,
                        )
                if with_bias:
                    brhs = (bv_sb[0:1, pairs[0] * 128:(pairs[0] + 1) * 128]
                            if len(pairs) == 1 else bv_sb[0:1, :])
                    for i in range(len(ks)):
                        nc.tensor.matmul(
                            accs[i][:, 0:n],
                            lhsT=ones512[0:1, 0:128],
                            rhs=brhs,
                            start=False,
                            stop=True,
                            skip_group_check=True,
                        )
                for i, k in enumerate(ks):
                    dst = (v_sb[:, k, pairs[0] * 128:(pairs[0] + 1) * 128]
                           if len(pairs) == 1 else v_sb[:, k, :])
                    nc.vector.tensor_copy(dst, accs[i][:, 0:n])

            zts = {}  # (p, j) -> zt tile
            es_tiles = {}
            pv_ps = {}

            def attn_scores(p, j, r0, r1):
                if r0 == 0:
                    es_tiles[(p, j)] = epool.tile([128, KC * 2 * 512], BF16, name="es")
                es = es_tiles[(p, j)]
                for c in range(r0, r1):
                    tp = c - 4 * j
                    a = 128 * tp if tp >= 0 else 0
                    ps = psP.tile([128, 1024], F32, name="ps_sc", bufs=2)
                    for hi in range(2):
                        # in SIM_SAFE mode h1 runs full-width on diagonal
                        # chunks so the exp never reads uninitialized PSUM;
                        # on HW the stripe is dead garbage (never consumed)
                        ah = a if (hi == 0 or not SIM_SAFE) else 0
                        prow = slice(64 * hi, 64 * hi + 64)
                        nc.tensor.matmul(
                            ps[:, 512 * hi + ah:512 * (hi + 1)],
                            lhsT=kt_sb[p][prow, c * 128:(c + 1) * 128],
                            rhs=qt_sb[p][prow, j * 512 + ah:(j + 1) * 512],
                            start=True,
                            stop=True,
                        )
                    nc.scalar.activation(
                        es[:, c * 1024 + a:(c + 1) * 1024],
                        ps[:, a:],
                        mybir.ActivationFunctionType.Exp,
                    )
                    if tp >= 0:
                        for hi in range(2):
                            sl = slice((c * 2 + hi) * 512 + a,
                                       (c * 2 + hi) * 512 + a + 128)
                            nc.vector.tensor_mul(
                                out=es[:, sl], in0=es[:, sl], in1=mask_sb[:, :],
                            )

            def attn_pv(p, j, r0, r1):
                nck = 4 * (j + 1)
                es = es_tiles[(p, j)]
                # PV + column sums; h0/h1 col-packed, emitted adjacently so
                # they run concurrently in disjoint array column groups
                if r0 == 0:
                    pv_ps[(p, j)] = (
                        psP.tile([128, 512], F32, name="ps_zs", bufs=2),
                        psP.tile([128, 512], F32, name="ps_zs", bufs=2),
                    )
                ps_z2, ps_s2 = pv_ps[(p, j)]
                ps_z = [ps_z2, ps_z2]
                ps_s = [ps_s2, ps_s2]
                for c in range(r0, r1):
                    tp = c - 4 * j
                    a = 128 * tp if tp >= 0 else 0
                    for hi in range(2):
                        col = 64 * hi
                        hcore = 2 * p + hi
                        nc.tensor.matmul(
                            ps_z[hi][col:col + 64, a:512],
                            lhsT=v_sb[:, c, hcore * DH:(hcore + 1) * DH],
                            rhs=es[:, (c * 2 + hi) * 512 + a:(c * 2 + hi + 1) * 512],
                            start=(c == 0),
                            stop=(c == nck - 1),
                            tile_position=(0, col),
                            skip_group_check=True,
                        )
                    for hi in range(2):
                        col = 64 * hi
                        nc.tensor.matmul(
                            ps_s[hi][col:col + 64, a:512],
                            lhsT=ones_bf[:, :],
                            rhs=es[:, (c * 2 + hi) * 512 + a:(c * 2 + hi + 1) * 512],
                            start=(c == 0),
                            stop=(c == nck - 1),
                            tile_position=(0, col),
                            skip_group_check=True,
                        )
                if r1 == nck:
                    recip = spool.tile([128, 512], F32, name="recip")
                    nc.vector.reciprocal_approx_fast(recip[:, :], ps_s2[:, :])
                    zt = ztpool.tile([128, 512], BF16, name=f"zt{p}{j}")
                    nc.vector.tensor_mul(zt[:, :], ps_z2[:, :], recip[:, :])
                    zts[(p, j)] = zt

            def emit_wo(j, d0, d1):
                for d in range(d0, d1):
                    ps = psP.tile([128, 512], F32, name="ps_acc", bufs=2)
                    for p in range(NPAIR):
                        nc.tensor.matmul(
                            ps[:, :],
                            lhsT=wo_sb[:, p, d * 128:(d + 1) * 128],
                            rhs=zts[(p, j)][:, :],
                            start=(p == 0),
                            stop=(p == NPAIR - 1),
                        )
                    ot = opool.tile([128, 512], F32, name="ot")
                    if j == 0 and d % 2 == 1:
                        nc.scalar.copy(ot[:, :], ps[:, :])
                    else:
                        nc.vector.tensor_copy(ot[:, :], ps[:, :])
                    eng = nc.gpsimd if d % 2 == 0 else nc.sync
                    eng.dma_start(
                        outT[d * 128:(d + 1) * 128, j * 512:(j + 1) * 512],
                        ot[:, :],
                    )

            # schedule: explicit interleave so the exp stream (Scalar) is
            # fed scores continuously while PV/projections/WO fill the PE,
            # ordered to chase the input DMA stream at the start.
            sc, pv = attn_scores, attn_pv
            qk_proj(0, 0)
            sc(0, 0, 0, 4)
            v_proj([0, 1], (0, 1))
            v_proj([2, 3], (0, 1))
            pv(0, 0, 0, 4)
            qk_proj(0, 1)
            sc(0, 1, 0, 2)
            sc(0, 1, 2, 4)
            v_proj([4, 5], (0, 1))
            sc(0, 1, 4, 6)
            v_proj([6, 7], (0, 1))
            sc(0, 1, 6, 8)
            pv(0, 1, 0, 4)
            qk_proj(0, 2)
            sc(0, 2, 0, 2)
            pv(0, 1, 4, 8)
            sc(0, 2, 2, 4)
            v_proj([8, 9], (0, 1))
            sc(0, 2, 4, 6)
            v_proj([10, 11], (0, 1))
            sc(0, 2, 6, 8)
            pv(0, 2, 0, 3)
            sc(0, 2, 8, 10)
            qk_proj(0, 3)
            sc(0, 2, 10, 12)
            pv(0, 2, 3, 6)
            sc(0, 3, 0, 2)
            pv(0, 2, 6, 9)
            sc(0, 3, 2, 4)
            pv(0, 2, 9, 12)
            sc(0, 3, 4, 6)
            v_proj([12, 13], (0, 1))
            sc(0, 3, 6, 8)
            v_proj([14, 15], (0, 1))
            sc(0, 3, 8, 10)
            qk_proj(1, 0)
            sc(0, 3, 10, 12)
            qk_proj(1, 1)
            sc(0, 3, 12, 14)
            pv(0, 3, 0, 3)
            sc(0, 3, 14, 16)
            pv(0, 3, 3, 6)
            qk_proj(1, 2)
            pv(0, 3, 6, 9)
            qk_proj(1, 3)
            sc(1, 3, 0, 2)
            pv(0, 3, 9, 12)
            sc(1, 3, 2, 4)
            pv(0, 3, 12, 16)
            sc(1, 3, 4, 6)
            pv(1, 3, 0, 2)
            sc(1, 3, 6, 8)
            pv(1, 3, 2, 4)
            sc(1, 3, 8, 10)
            pv(1, 3, 4, 6)
            sc(1, 3, 10, 12)
            pv(1, 3, 6, 8)
            sc(1, 3, 12, 14)
            pv(1, 3, 8, 10)
            sc(1, 3, 14, 16)
            pv(1, 3, 10, 13)
            sc(1, 2, 0, 2)
            pv(1, 3, 13, 16)
            sc(1, 2, 2, 4)
            emit_wo(3, 0, 3)
            sc(1, 2, 4, 6)
            emit_wo(3, 3, 6)
            sc(1, 2, 6, 8)
            emit_wo(3, 6, 8)
            sc(1, 2, 8, 10)
            pv(1, 2, 0, 3)
            sc(1, 2, 10, 12)
            pv(1, 2, 3, 6)
            sc(1, 1, 0, 2)
            pv(1, 2, 6, 9)
            sc(1, 1, 2, 4)
            pv(1, 2, 9, 12)
            sc(1, 1, 4, 6)
            emit_wo(2, 0, 3)
            sc(1, 1, 6, 8)
            emit_wo(2, 3, 6)
            sc(1, 0, 0, 2)
            emit_wo(2, 6, 8)
            sc(1, 0, 2, 4)
            pv(1, 1, 0, 4)
            pv(1, 1, 4, 8)
            emit_wo(1, 0, 4)
            pv(1, 0, 0, 4)
            emit_wo(1, 4, 8)
            emit_wo(0, 0, 8)

    nc.compile()
    return nc


_cache = {}


def _get(with_bias: bool):
    if with_bias not in _cache:
        _cache[with_bias] = _build(with_bias)
    return _cache[with_bias]


def kernel(x, W_Q, W_K, W_V, W_O, b_Q, b_K, b_V, b_O):
    global LAST_EXEC_NS
    x = np.asarray(x, dtype=np.float32)
    W_Q = np.asarray(W_Q, dtype=np.float32)
    W_K = np.asarray(W_K, dtype=np.float32)
    W_V = np.asarray(W_V, dtype=np.float32)
    W_O = np.asarray(W_O, dtype=np.float32)
    b_Q = np.asarray(b_Q, dtype=np.float32)
    b_K = np.asarray(b_K, dtype=np.float32)
    b_V = np.asarray(b_V, dtype=np.float32)
    b_O = np.asarray(b_O, dtype=np.float32)

    with_bias = bool(np.any(b_Q) or np.any(b_K) or np.any(b_V))
    nc = _get(with_bias)

    bf = ml_dtypes.bfloat16

    xT = np.ascontiguousarray(x.transpose(0, 2, 1))  # [B, DM, S]
    # chunk-major [B, 128, DC, S] bf16
    xc = np.ascontiguousarray(
        xT.reshape(B, DC, 128, S).transpose(0, 2, 1, 3)
    ).astype(bf)
    kp = np.arange(128)[:, None]
    qf = np.arange(128)[None, :]
    mask = np.where(qf >= kp, 1.0, 0.0).astype(bf)

    def pack_w(w):  # [DM, NH*DH] -> [128, NPAIR, DC*128] bf16, pair-major
        w = w.reshape(DC, 128, NPAIR, 128)           # [c, p, pair, m]
        w = w.transpose(1, 2, 0, 3)                  # [p, pair, c, m]
        return np.ascontiguousarray(w.reshape(128, NPAIR, DC, 128)).astype(bf)

    in_maps = []
    for core in range(8):
        b, g = divmod(core, 4)
        hs = slice(NH * g, NH * g + NH)

        m = {
            "wq": pack_w((W_Q[hs] * 0.125).transpose(1, 0, 2).reshape(DM, NH * DH)),
            "wk": pack_w(W_K[hs].transpose(1, 0, 2).reshape(DM, NH * DH)),
            "wv": pack_w(W_V[hs].transpose(1, 0, 2).reshape(DM, NH * DH)),
            "wo": np.ascontiguousarray(
                W_O[hs].reshape(NH * DH, DM).astype(bf)
                .reshape(NPAIR, 128, DM).transpose(1, 0, 2).reshape(128, NPAIR * DM)
            ),
            "mask": mask,
        }
        for i in range(3):
            m[f"xb{i}"] = np.ascontiguousarray(
                xc[b][:, :, BLK0[i]:BLK0[i] + BLKS[i]]
            )
        if with_bias:
            m["bq"] = (b_Q[hs] * 0.125).reshape(1, NH * DH).astype(bf)
            m["bk"] = b_K[hs].reshape(1, NH * DH).astype(bf)
            m["bv"] = b_V[hs].reshape(1, NH * DH).astype(bf)
        in_maps.append(m)

    kwargs = {}
    if TRACE:
        kwargs = {"trace": True}
        if TRACE_DIR:
            kwargs["tmpdir"] = TRACE_DIR
    res = run_bass_kernel_spmd(nc, in_maps, list(range(8)), **kwargs)
    LAST_EXEC_NS = res.exec_time_ns

    out = np.empty((B, S, DM), dtype=np.float32)
    for b in range(B):
        acc = res.results[4 * b]["outT"].astype(np.float32)
        for g in range(1, 4):
            acc = acc + res.results[4 * b + g]["outT"]
        out[b] = acc.T + b_O[None, :]
    return out
